# revision 5
# baseline (speedup 1.0000x reference)
"""Trainium2 Bass kernel for DFlashAttentionV5.

Reference computation (fp32, single device):
    Q/K/V/Kctx/Vctx projections -> rmsnorm(Q), rmsnorm(K_full) -> softmax
    attention over concat(ctx, self) keys/values -> output projection.

Sharding over 8 NeuronCores: batch (2-way) x head-group (4-way).
Core c handles batch b = c // 4 and the 4 heads 4*g..4*g+3 (g = c % 4).

Each core computes its 4 heads' attention, then the FULL-WIDTH (2048-col)
output-projection partial from its local heads, accumulated in PSUM across
heads.  A ReduceScatter over the 4-core batch group sums the partials and
leaves each core with two 128-query slices of the final output (no per-head
all-gathers).  The RS runs in two query-half chunks so the first one
overlaps the tail of the attention compute.

Schedule (per core):
  s0: self Q/K/V projections for heads 0,1          (x resident)
  s1: self projections heads 2,3                    (x freed after)
  s2: ctx K/V projections head 0                    (ctx halves resident)
  s3: ctx head 1 + attention head 0 interleaved
  s4: ctx head 2 + attention head 1
  s5: ctx head 3 + attention head 2
  end: attention head 3 (qh0 units, out-proj wave qh0 interleaved with qh1
       units), ReduceScatter(qh0), wave qh1, ReduceScatter(qh1)

Attention is emitted in (query-half, key-chunk) units with the PV matmul
lagged one unit behind its scores matmul so the PE never waits on the
Activation engine's exp.

All matmuls run in bf16 (fp32 PSUM accumulation); softmax statistics in
fp32.  x/ctx arrive pre-transposed from the host ([D, tokens]); all weights
arrive pre-shuffled so every DMA moves >=2KB contiguous runs.

Self-contained: hardcodes all shapes; only imports concourse + numpy.
"""

import math

import numpy as np
import ml_dtypes

import concourse.bass as bass
import concourse.mybir as mybir
import concourse.tile as tile
from concourse.bass_utils import run_bass_kernel_spmd

BF16 = mybir.dt.bfloat16
F32 = mybir.dt.float32
AF = mybir.ActivationFunctionType
ALU = mybir.AluOpType

# Problem dims
B, K, CTX, D, H, HD = 2, 1024, 2048, 2048, 16, 128
S = CTX + K            # 3072 keys per query
NCORES = 8
GROUPS = 4             # head groups (tensor-parallel within a batch)
NH = H // GROUPS       # 4 local heads per core
E = H * HD             # 2048
EW = NH * HD           # 512 local attention width
DCH = D // 128         # 16 contraction chunks
SCH = S // 128         # 24 key chunks
CCH = CTX // 128       # 16 ctx key chunks
QH = 512               # query-half width (softmax / RS granularity)
NQH = K // QH          # 2
SCALE = 1.0 / math.sqrt(HD)
EPS = 1e-6
REPLICA_GROUPS = [[0, 1, 2, 3], [4, 5, 6, 7]]

_CACHE = {}


def _build(with_mask: bool):
    """Build the SPMD bass program (same program on all 8 cores)."""
    nc = bass.Bass(num_devices=NCORES)

    xT_d = nc.declare_dram_parameter("xT", [D, K], BF16, isOutput=False)
    cT_d = nc.declare_dram_parameter("cT", [D, CTX], BF16, isOutput=False)
    # weights pre-shuffled on host: rows h*128+p hold
    # w[dch*128+p, g*EW + h*128 + cl] at col dch*128+cl.
    wq_d = nc.declare_dram_parameter("wq", [NH * 128, D], BF16, isOutput=False)
    wk_d = nc.declare_dram_parameter("wk", [NH * 128, D], BF16, isOutput=False)
    wv_d = nc.declare_dram_parameter("wv", [NH * 128, D], BF16, isOutput=False)
    wck_d = nc.declare_dram_parameter("wck", [NH * 128, D], BF16,
                                      isOutput=False)
    wcv_d = nc.declare_dram_parameter("wcv", [NH * 128, D], BF16,
                                      isOutput=False)
    # out-proj rows for local heads: row h*128+p = w_out[g*EW + h*128 + p, :]
    wo_d = nc.declare_dram_parameter("wo", [NH * 128, E], BF16, isOutput=False)
    qnw_d = nc.declare_dram_parameter("qnw", [HD, 1], F32, isOutput=False)
    knw_d = nc.declare_dram_parameter("knw", [HD, 1], F32, isOutput=False)
    if with_mask:
        mt_d = nc.declare_dram_parameter("maskT", [S, K], F32, isOutput=False)
    # final output: this core's two 128-query slices (one per RS chunk)
    out_d = nc.declare_dram_parameter("out", [2 * 128, E], BF16, isOutput=True)

    with tile.TileContext(nc, num_cores=NCORES) as tc:
        with (
            tc.tile_pool(name="const", bufs=1) as constp,
            tc.tile_pool(name="perm", bufs=1) as perm,
            tc.tile_pool(name="stat", bufs=2) as statp,
            tc.tile_pool(name="fin", bufs=2) as finp,
            tc.tile_pool(name="bc", bufs=2) as bcp,
            tc.tile_pool(name="sq", bufs=2) as sqp,
            tc.tile_pool(name="pT", bufs=3) as pTp,
            tc.tile_pool(name="acc", bufs=3) as accp,
            tc.tile_pool(name="w", bufs=2) as wpool,
            tc.tile_pool(name="psP", bufs=2, space="PSUM") as psP,
            tc.tile_pool(name="psS", bufs=2, space="PSUM") as psS,
            tc.tile_pool(name="psPV", bufs=2, space="PSUM") as psPV,
            tc.tile_pool(name="ps1", bufs=2, space="PSUM") as ps1,
            tc.tile_pool(name="dram", bufs=1, space="DRAM") as dram,
        ):
            ones_col = constp.tile([128, 1], BF16)
            nc.any.memset(ones_col, 1.0)
            ones_row = constp.tile([1, 128], BF16)
            nc.any.memset(ones_row, 1.0)
            qnw_sb = constp.tile([HD, 1], F32)
            knw_sb = constp.tile([HD, 1], F32)

            # Resident tensors (bf16):
            K_sb = [perm.tile([128, S], BF16, tag=f"K{h}", bufs=1, name=f"K{h}")
                    for h in range(NH)]
            # V slice (s, h) at cols s*EW + h*128; partitions = tokens of s
            V_all = perm.tile([128, SCH * EW], BF16, tag="Vall", bufs=1,
                              name="Vall")
            QT_sb = [perm.tile([128, K], BF16, tag=f"Q{h}", bufs=1, name=f"Q{h}")
                     for h in range(NH)]
            attnT = [perm.tile([128, K], BF16, tag=f"A{h}", bufs=1,
                               name=f"at{h}") for h in range(NH)]

            partial_d = dram.tile([K, E], BF16, name="partial")
            rsout_d = dram.tile([2 * 128, E], BF16, name="rsout")

            if with_mask:
                maskp = pTp  # reuse pool with a distinct tag

            # ---------------- helper chains ----------------
            def rms_norm_T(ps, dest_ap, nw_sb):
                """psum [128=hd, 512=tokens] fp32 -> dest bf16, rmsnorm over
                the 128 partitions (head dim) per token."""
                sqt = sqp.tile([128, QH], BF16, tag="sq")
                nc.scalar.square(sqt[:], ps[:])
                ps_s = ps1.tile([128, QH], F32, tag="ps1")
                nc.tensor.matmul(ps_s[0:1, :], ones_col[:], sqt[:],
                                 start=True, stop=True)
                mean = statp.tile([1, QH], F32, tag="mean")
                nc.vector.tensor_scalar(mean[:], ps_s[0:1, :], 1.0 / HD, EPS,
                                        ALU.mult, ALU.add)
                rec = statp.tile([1, QH], F32, tag="rec")
                nc.vector.reciprocal(rec[:], mean[:])
                rs = statp.tile([1, QH], BF16, tag="rs")
                nc.scalar.sqrt(rs[:], rec[:])  # rsqrt = sqrt(1/x)
                ps_b = ps1.tile([128, QH], F32, tag="ps1")
                nc.tensor.matmul(ps_b[:], ones_row[:], rs[:],
                                 start=True, stop=True)
                bct = bcp.tile([128, QH], F32, tag="bc")
                nc.scalar.copy(bct[:], ps_b[:])
                nc.vector.scalar_tensor_tensor(dest_ap, ps[:], nw_sb[:],
                                               bct[:], ALU.mult, ALU.mult)

            # ---------------- projection groups ----------------
            def load_w(w_d, h, tag):
                wt = wpool.tile([128, D], BF16, tag=tag, name=f"{tag}{h}")
                nc.sync.dma_start(wt[:], w_d[h * 128:(h + 1) * 128, :])
                return wt

            def qk_group(wt, src, s_off, dest, dest_off, nw_sb):
                """One 512-token QT/K projection group.
                src: list of 4 srcT tiles [128, 4*1024]; s_off: token offset
                within the 4096-token window (0..3584, mult of 512)."""
                ps = psP.tile([128, QH], F32, tag="pj")
                for d in range(DCH):
                    st = src[d // 4]
                    off = (d % 4) * 1024 + s_off
                    nc.tensor.matmul(ps[:], wt[:, d * 128:(d + 1) * 128],
                                     st[:, off:off + QH],
                                     start=(d == 0), stop=(d == DCH - 1))
                rms_norm_T(ps, dest[:, dest_off:dest_off + QH], nw_sb)

            def v_group(wt, h, src, s_off, s_base):
                """One 512-token (4 s-chunk) V projection group.
                Writes V_all slices (s_base+t, h) for t in 0..3."""
                ps = psP.tile([128, 512], F32, tag="pj")
                for t in range(4):
                    for d in range(DCH):
                        st = src[d // 4]
                        off = (d % 4) * 1024 + s_off + t * 128
                        nc.tensor.matmul(
                            ps[:, t * 128:(t + 1) * 128],
                            st[:, off:off + 128],
                            wt[:, d * 128:(d + 1) * 128],
                            start=(d == 0), stop=(d == DCH - 1))
                dst = V_all[:].rearrange("p (s hh q) -> p s hh q",
                                         hh=NH, q=128)[:, s_base:s_base + 4,
                                                       h:h + 1, :]
                src_ap = ps[:].rearrange("p (a o q) -> p a o q", o=1, q=128)
                nc.vector.tensor_copy(dst, src_ap)

            # ---------------- attention units ----------------
            # per-(head, qhalf) state: pv psum, f32 denominator acc, pT tiles
            st_pv = {}
            st_acc = {}
            st_pT = {}

            def sc_part(h, qh, s):
                """scores + exp + denominator accumulate for one unit."""
                key = (h, qh)
                sc = psS.tile([128, QH], F32, tag="psS")
                nc.tensor.matmul(sc[:], K_sb[h][:, s * 128:(s + 1) * 128],
                                 QT_sb[h][:, qh * QH:(qh + 1) * QH],
                                 start=True, stop=True)
                if with_mask:
                    mrow = maskp.tile([128, QH], F32, tag="mrow")
                    nc.sync.dma_start(
                        mrow[:], mt_d[s * 128:(s + 1) * 128,
                                      qh * QH:(qh + 1) * QH])
                    nc.vector.tensor_tensor(sc[:], sc[:], mrow[:], ALU.add)
                pT = pTp.tile([128, QH], BF16, tag="pT")
                nc.scalar.activation(pT[:], sc[:], AF.Exp, scale=SCALE)
                if s == 0:
                    st_acc[key] = accp.tile([128, QH], F32, tag="acc",
                                            name=f"ac{h}_{qh}")
                    nc.vector.tensor_copy(st_acc[key][:], pT[:])
                else:
                    nc.vector.tensor_tensor(st_acc[key][:], st_acc[key][:],
                                            pT[:], ALU.add)
                st_pT[(h, qh, s)] = pT

            def pv_part(h, qh, s):
                key = (h, qh)
                pT = st_pT.pop((h, qh, s))
                if s == 0:
                    st_pv[key] = psPV.tile([128, QH], F32, tag="pv",
                                           name=f"pv{h}_{qh}")
                nc.tensor.matmul(st_pv[key][:],
                                 V_all[:, s * EW + h * 128:
                                       s * EW + (h + 1) * 128],
                                 pT[:], start=(s == 0), stop=(s == SCH - 1))

            def finish_head(h, qh):
                """denominator -> broadcast -> normalized attnT chunk."""
                key = (h, qh)
                acc = st_acc.pop(key)
                pv = st_pv.pop(key)
                accb = finp.tile([128, QH], BF16, tag="accb")
                nc.vector.tensor_copy(accb[:], acc[:])
                ps_d = ps1.tile([128, QH], F32, tag="ps1")
                nc.tensor.matmul(ps_d[0:1, :], ones_col[:], accb[:],
                                 start=True, stop=True)
                rec = finp.tile([1, QH], F32, tag="rec2")
                nc.vector.reciprocal(rec[:], ps_d[0:1, :])
                rb = finp.tile([1, QH], BF16, tag="rb")
                nc.vector.tensor_copy(rb[:], rec[:])
                ps_b = ps1.tile([128, QH], F32, tag="ps1")
                nc.tensor.matmul(ps_b[:], ones_row[:], rb[:],
                                 start=True, stop=True)
                bct = bcp.tile([128, QH], F32, tag="bc")
                nc.scalar.copy(bct[:], ps_b[:])
                nc.vector.tensor_tensor(attnT[h][:, qh * QH:(qh + 1) * QH],
                                        pv[:], bct[:], ALU.mult)

            def attn_emitters(h):
                """One closure per (qh, s) unit; PV lagged one unit."""
                units = [(qh, s) for qh in range(NQH) for s in range(SCH)]
                ems = []
                for i, (qh, s) in enumerate(units):
                    def f(i=i, qh=qh, s=s):
                        sc_part(h, qh, s)
                        if i > 0:
                            pqh, psv = units[i - 1]
                            pv_part(h, pqh, psv)
                            if psv == SCH - 1:
                                finish_head(h, pqh)
                        if i == len(units) - 1:
                            pv_part(h, qh, s)
                            finish_head(h, qh)
                    ems.append(f)
                return ems

            def interleave(groups, units):
                """Emit proj groups with attention units spread between."""
                ng, nu = len(groups), len(units)
                if nu == 0:
                    for g in groups:
                        g()
                    return
                if ng == 0:
                    for u in units:
                        u()
                    return
                ui = 0
                for i, g in enumerate(groups):
                    g()
                    target = (i + 1) * nu // ng
                    while ui < target:
                        units[ui]()
                        ui += 1

            # ---------------- emission ----------------
            nc.sync.dma_start(qnw_sb[:], qnw_d[:])
            nc.sync.dma_start(knw_sb[:], knw_d[:])

            with tc.tile_pool(name="srcT", bufs=1) as srcTp:
                setA = [srcTp.tile([128, 4096], BF16, tag=f"sa{i}", bufs=1,
                                   name=f"xT{i}") for i in range(4)]
                setB = [srcTp.tile([128, 4096], BF16, tag=f"sb{i}", bufs=1,
                                   name=f"cTa{i}") for i in range(4)]

                def load_set(grp, dram_ap):
                    for i in range(4):
                        nc.sync.dma_start(
                            grp[i][:].rearrange("p (a t) -> p a t", t=1024),
                            dram_ap[i * 512:(i + 1) * 512, :]
                            .rearrange("(a p) t -> p a t", p=128))

                wq_t = [None] * NH
                wk_t = [None] * NH
                wv_t = [None] * NH
                wck_t = [None] * NH
                wcv_t = [None] * NH

                def self_groups(h):
                    gs = []
                    for j in range(2):      # Q tokens j*512
                        gs.append(lambda j=j: qk_group(
                            wq_t[h], setA, j * QH, QT_sb[h], j * QH, qnw_sb))
                    for j in range(2):      # K self -> K_sb cols CTX+...
                        gs.append(lambda j=j: qk_group(
                            wk_t[h], setA, j * QH, K_sb[h], CTX + j * QH,
                            knw_sb))
                    for j in range(2):      # V self -> s chunks 16..23
                        gs.append(lambda j=j: v_group(
                            wv_t[h], h, setA, j * QH, CCH + j * 4))
                    return gs

                def ctx_groups(h, setC):
                    gs = []
                    for j in range(2):      # K ctx half0
                        gs.append(lambda j=j: qk_group(
                            wck_t[h], setB, j * QH, K_sb[h], j * QH, knw_sb))
                    for j in range(2):      # V ctx half0 -> s 0..7
                        gs.append(lambda j=j: v_group(
                            wcv_t[h], h, setB, j * QH, j * 4))
                    for j in range(2):      # K ctx half1
                        gs.append(lambda j=j: qk_group(
                            wck_t[h], setC, j * QH, K_sb[h], 1024 + j * QH,
                            knw_sb))
                    for j in range(2):      # V ctx half1 -> s 8..15
                        gs.append(lambda j=j: v_group(
                            wcv_t[h], h, setC, j * QH, 8 + j * 4))
                    return gs

                # --- stage 0: self projections heads 0,1 ---
                wq_t[0] = load_w(wq_d, 0, "wq")
                load_set(setA, xT_d)
                wk_t[0] = load_w(wk_d, 0, "wk")
                wv_t[0] = load_w(wv_d, 0, "wv")
                wq_t[1] = load_w(wq_d, 1, "wq")
                wk_t[1] = load_w(wk_d, 1, "wk")
                wv_t[1] = load_w(wv_d, 1, "wv")
                g01 = self_groups(0) + self_groups(1)
                for g in g01[:6]:
                    g()
                load_set(setB, cT_d[:, 0:1024])
                wq_t[2] = load_w(wq_d, 2, "wq")
                wk_t[2] = load_w(wk_d, 2, "wk")
                wv_t[2] = load_w(wv_d, 2, "wv")
                for g in g01[6:]:
                    g()

                # --- stage 1: self projections heads 2,3 ---
                wq_t[3] = load_w(wq_d, 3, "wq")
                wk_t[3] = load_w(wk_d, 3, "wk")
                wv_t[3] = load_w(wv_d, 3, "wv")
                wck_t[0] = load_w(wck_d, 0, "wck")
                wcv_t[0] = load_w(wcv_d, 0, "wcv")
                g23 = self_groups(2) + self_groups(3)
                for g in g23:
                    g()

                # xT slots free; ctx half1 reuses their tags
                setC = [srcTp.tile([128, 4096], BF16, tag=f"sa{i}", bufs=1,
                                   name=f"cTb{i}") for i in range(4)]
                load_set(setC, cT_d[:, 1024:2048])
                wck_t[1] = load_w(wck_d, 1, "wck")
                wcv_t[1] = load_w(wcv_d, 1, "wcv")

                # --- stage 2: ctx head 0 (no attention yet) ---
                interleave(ctx_groups(0, setC), [])

                # --- stages 3-5: ctx head h+1 + attention head h ---
                for h in range(NH - 1):
                    if h + 2 < NH:
                        wck_t[h + 2] = load_w(wck_d, h + 2, "wck")
                        wcv_t[h + 2] = load_w(wcv_d, h + 2, "wcv")
                    interleave(ctx_groups(h + 1, setC), attn_emitters(h))

            # ---------------- endgame: attn(h3) + waves + RS ----------------
            with (
                tc.tile_pool(name="wo", bufs=1) as wop,
                tc.tile_pool(name="stg", bufs=2) as stgp,
            ):
                wo_t = [wop.tile([128, E], BF16, tag=f"wo{h}", bufs=1,
                                 name=f"wo{h}") for h in range(NH)]
                for h in range(NH):
                    nc.sync.dma_start(wo_t[h][:], wo_d[h * 128:(h + 1) * 128, :])

                def wave(qh):
                    """out-proj partial for query-half qh: psum accumulates
                    the 4 heads; one Act copy + DMA per 128-query chunk."""
                    for t in range(4):
                        q0 = qh * QH + t * 128
                        stg = stgp.tile([128, E], BF16, tag="stg")
                        for cg in range(4):
                            ps = ps1.tile([128, QH], F32, tag="ps1")
                            for h in range(NH):
                                nc.tensor.matmul(
                                    ps[:], attnT[h][:, q0:q0 + 128],
                                    wo_t[h][:, cg * QH:(cg + 1) * QH],
                                    start=(h == 0), stop=(h == NH - 1))
                            if cg % 2 == 0:
                                nc.scalar.copy(stg[:, cg * QH:(cg + 1) * QH],
                                               ps[:])
                            else:
                                nc.vector.tensor_copy(
                                    stg[:, cg * QH:(cg + 1) * QH], ps[:])
                        nc.sync.dma_start(partial_d[q0:q0 + 128, :], stg[:])

                last_units = attn_emitters(NH - 1)
                for em in last_units[:SCH]:       # qh0 units (+ finish)
                    em()
                # wave(0) interleaved with qh1 units
                qh1 = last_units[SCH:]
                for em in qh1[:6]:
                    em()
                wave(0)
                for em in qh1[6:]:
                    em()

                def rs(qh):
                    nc.gpsimd.collective_compute(
                        "ReduceScatter", ALU.add,
                        replica_groups=REPLICA_GROUPS,
                        ins=[partial_d[qh * QH:(qh + 1) * QH, :].opt()],
                        outs=[rsout_d[qh * 128:(qh + 1) * 128, :].opt()],
                    )
                    nc.sync.dma_start(out_d[qh * 128:(qh + 1) * 128, :],
                                      rsout_d[qh * 128:(qh + 1) * 128, :])

                rs(0)
                wave(1)
                rs(1)

    return nc


def _split_multiwaits(nc):
    """walrus codegen in this container rejects instructions with more than
    one semaphore wait; split the excess onto preceding NoOps on the same
    engine."""
    for f in nc.m.functions:
        for blk in f.blocks:
            idx = 0
            while idx < len(blk.instructions):
                inst = blk.instructions[idx]
                si = inst.sync_info
                maxw = 1
                if si is None or len(si.on_wait) <= maxw:
                    idx += 1
                    continue
                waits = list(si.on_wait)
                ncarry = (len(waits) - 1) // maxw  # leave <=maxw on inst
                for k in range(ncarry):
                    chunk = waits[k * maxw:(k + 1) * maxw]
                    nop = mybir.InstNoOp(
                        name=nc.get_next_instruction_name(),
                        ins=[], outs=[],
                        bass_nofuse=True,
                        sync_info=mybir.SyncInfo(on_wait=chunk, on_update=[]),
                    )
                    nop.engine = inst.engine
                    nc.register_instruction(nop)
                    blk.instructions.insert(idx, nop)
                    idx += 1
                si.on_wait = waits[ncarry * maxw:]
                idx += 1


def _get_program(with_mask: bool):
    key = ("prog", with_mask)
    if key not in _CACHE:
        nc = _build(with_mask)
        _split_multiwaits(nc)
        _CACHE[key] = nc
    return _CACHE[key]


def kernel(x, context, attn_mask, w_q, w_k, w_v, w_ctx_k, w_ctx_v, w_out,
           q_norm_w, k_norm_w):
    x = np.asarray(x, np.float32)
    context = np.asarray(context, np.float32)
    attn_mask = np.asarray(attn_mask, np.float32)
    w_q = np.asarray(w_q, np.float32)
    w_k = np.asarray(w_k, np.float32)
    w_v = np.asarray(w_v, np.float32)
    w_ctx_k = np.asarray(w_ctx_k, np.float32)
    w_ctx_v = np.asarray(w_ctx_v, np.float32)
    w_out = np.asarray(w_out, np.float32)
    q_norm_w = np.asarray(q_norm_w, np.float32)
    k_norm_w = np.asarray(k_norm_w, np.float32)

    with_mask = bool(np.any(attn_mask))
    nc = _get_program(with_mask)
    in_maps = _prepare_in_maps(x, context, attn_mask, w_q, w_k, w_v, w_ctx_k,
                               w_ctx_v, w_out, q_norm_w, k_norm_w, with_mask)

    res = run_bass_kernel_spmd(nc, in_maps, list(range(NCORES))).results
    return _assemble(res)


def _assemble(res):
    out = np.empty((B, K, D), np.float32)
    for c in range(NCORES):
        b, r = c // GROUPS, c % GROUPS
        o = np.asarray(res[c]["out"], dtype=np.float32)
        out[b, r * 128:(r + 1) * 128, :] = o[0:128]
        out[b, QH + r * 128:QH + (r + 1) * 128, :] = o[128:256]
    return out


def _shuffle_w(w, g):
    """[D, E] col-slice for group g -> [NH*128, D] where row h*128+p,
    col dch*128+cl = w[dch*128+p, g*EW + h*128 + cl]."""
    ws = w[:, g * EW:(g + 1) * EW]                     # [D, 512]
    ws = ws.reshape(DCH, 128, NH, 128)                 # [dch, p, h, cl]
    return np.ascontiguousarray(
        ws.transpose(2, 1, 0, 3).reshape(NH * 128, D)).astype(
            ml_dtypes.bfloat16)


def _prepare_in_maps(x, context, attn_mask, w_q, w_k, w_v, w_ctx_k, w_ctx_v,
                     w_out, q_norm_w, k_norm_w, with_mask):
    bf16 = ml_dtypes.bfloat16
    xT = [np.ascontiguousarray(x[b].T).astype(bf16) for b in range(B)]
    cT = [np.ascontiguousarray(context[b].T).astype(bf16) for b in range(B)]
    in_maps = []
    for c in range(NCORES):
        b, g = c // GROUPS, c % GROUPS
        m = {
            "xT": xT[b],
            "cT": cT[b],
            "wq": _shuffle_w(w_q, g),
            "wk": _shuffle_w(w_k, g),
            "wv": _shuffle_w(w_v, g),
            "wck": _shuffle_w(w_ctx_k, g),
            "wcv": _shuffle_w(w_ctx_v, g),
            "wo": np.ascontiguousarray(
                w_out[g * EW:(g + 1) * EW, :]).astype(bf16),
            "qnw": q_norm_w.reshape(HD, 1).astype(np.float32).copy(),
            "knw": k_norm_w.reshape(HD, 1).astype(np.float32).copy(),
        }
        if with_mask:
            # mask [B,1,K,S] -> transposed [S,K] per batch (fp32), pre-divided
            # by SCALE since the kernel folds SCALE into the exp activation.
            m["maskT"] = np.ascontiguousarray(attn_mask[b, 0].T) * (1.0 / SCALE)
        in_maps.append(m)
    return in_maps
